# revision 39
# baseline (speedup 1.0000x reference)
"""Trainium2 Bass kernel: sparse 7x7x7 stride-1 max-pool over a 64^3 voxel grid
(MinkowskiEngine semantics) + per-point MLP (1x1 conv -> ReLU -> 1x1 conv ->
sigmoid) * feats.

Strategy (8 NeuronCores, SPMD, no collectives):
  - Shard the dense grid along z: core k owns z in [8k, 8k+8), processes a
    14-plane z-slab (3-voxel halo each side, replicated -> no exchange).
  - HOST pre-builds the dense, channel-major (transposed) fp16 planes:
    planes[x] = [2, 128, 64*14] with empty voxels = SENT. The device does
    ZERO scatter / transpose / indirect-DMA work: x-planes arrive by direct
    DMA into SBUF already in [channel-part, (y,z)-free] layout.
  - Separable windowed max (7 = overlap of two 4-windows -> 3 tensor_max
    per axis), all on DVE in fp16 (2x mode; max is order-preserving).
    The z- and y-passes are batched 2 x-planes per instruction (5D APs) to
    amortize instruction overhead. GPSIMD is NOT used for elementwise work:
    its SBUF port is shared with DVE and halves DVE throughput.
  - Per-plane MLP without any transposes: hp = sum_h W1[h].T @ px[h] (R on
    partitions), then y2[h] = W2[:, h-block].T @ relu(hp) which lands
    directly back in [channel-part, vox] layout. Sigmoid on ACT.
  - out = P_orig * sig computed densely on DVE (fp16), written per plane
    pair; host gathers the occupied rows and casts to fp32 (pure unshard).

Measured on the 8-core axon TRN2 fleet: HW exec ~429 us (baseline 870-900 us),
rel err ~8.3e-4 vs the fp32 reference.
"""

from contextlib import ExitStack
from dataclasses import dataclass

import numpy as np

C = 256
R = 128
SENT = -100.0  # < min(feats) ~ -5.6; keeps the fp16 MLP finite at empty voxels


@dataclass(frozen=True)
class Cfg:
    D: int = 64           # grid extent per axis
    ZS: int = 8           # owned z-planes per core
    NPTS: int = 100000    # total points
    ncores: int = 8

    @property
    def ZH(self):
        return self.ZS + 6

    @property
    def YZ(self):
        return self.D * self.ZH     # 896

    @property
    def NX(self):
        return self.D

    @property
    def VOXH(self):
        return self.D * self.ZS     # 512 owned voxels per x-plane

    @property
    def YP(self):
        return self.D + 6           # padded y extent


FULL = Cfg()


def build_nc(cfg: Cfg):
    """Build the (SPMD, per-core-identical) Bass program."""
    import concourse.bacc as bacc
    import concourse.tile as tile
    from concourse import mybir

    AF = mybir.ActivationFunctionType
    f32 = mybir.dt.float32
    dt = mybir.dt.float16

    D, ZS, ZH, YZ, NX = cfg.D, cfg.ZS, cfg.ZH, cfg.YZ, cfg.NX
    VOXH = cfg.VOXH
    VOX2 = 2 * VOXH           # 1024
    YP = cfg.YP
    NG = NX // 2              # 32 groups of 2 x-planes

    nc = bacc.Bacc("TRN2", target_bir_lowering=False, debug=False,
                   enable_asserts=False, num_devices=cfg.ncores)

    planes = nc.dram_tensor("planes", [NX * 2 * 128, YZ], dt,
                            kind="ExternalInput").ap()
    w1 = nc.dram_tensor("w1", [C, R], dt, kind="ExternalInput").ap()
    w2 = nc.dram_tensor("w2", [R, C], dt, kind="ExternalInput").ap()
    outd = nc.dram_tensor("out", [NX * 2 * 128, VOXH], dt,
                          kind="ExternalOutput").ap()

    with tile.TileContext(nc) as tc, ExitStack() as ctx:
        const = ctx.enter_context(tc.tile_pool(name="const", bufs=1))
        pp = ctx.enter_context(tc.tile_pool(name="pp", bufs=6))
        z2p = ctx.enter_context(tc.tile_pool(name="z2p", bufs=1))
        z4p = ctx.enter_context(tc.tile_pool(name="z4p", bufs=1))
        y4p = ctx.enter_context(tc.tile_pool(name="y4p", bufs=1))
        pxp = ctx.enter_context(tc.tile_pool(name="pxp", bufs=3))
        hpp = ctx.enter_context(tc.tile_pool(name="hpp", bufs=2, space="PSUM"))
        y2p = ctx.enter_context(tc.tile_pool(name="y2p", bufs=3, space="PSUM"))
        hsp = ctx.enter_context(tc.tile_pool(name="hsp", bufs=3))
        sgp = ctx.enter_context(tc.tile_pool(name="sgp", bufs=3))
        otp = ctx.enter_context(tc.tile_pool(name="otp", bufs=3))

        # ---- first plane pair prefetched (per-plane, so the z-pass can
        # start after half the transfer) ahead of everything else
        P2_first = pp.tile([128, 2 * 2 * YZ], dt)
        for x in (0, 1):
            nc.sync.dma_start(
                P2_first[:, x * 2 * YZ:(x + 1) * 2 * YZ].rearrange(
                    "p (h v) -> p h v", h=2),
                planes[x * 256:(x + 1) * 256, :].rearrange(
                    "(h p) v -> p h v", p=128),
            )

        # ---- constants
        w1sb = const.tile([128, 2 * R], dt)
        nc.sync.dma_start(
            w1sb[:].rearrange("p (h r) -> p h r", h=2),
            w1.rearrange("(h p) r -> p h r", p=128),
        )
        w2sb = const.tile([128, C], dt)
        nc.sync.dma_start(w2sb[:], w2)
        neg = const.tile([128, VOX2], dt)
        nc.gpsimd.memset(neg[:], SENT)
        # persistent y-padded buffer (2 planes wide, z-major); borders once
        ypad = const.tile([128, 2 * 2 * ZS * YP], dt)
        ypv = ypad[:].rearrange("p (x h a b) -> p x h a b", x=2, h=2, a=ZS)
        nc.gpsimd.memset(ypv[:, :, :, :, 0:3], SENT)
        nc.gpsimd.memset(ypv[:, :, :, :, D + 3:D + 6], SENT)
        # persistent m2y buffer: cols {0,1,67,68} are always SENT (both
        # window inputs are padding there) - memset once, never recompute
        m2yc = const.tile([128, 2 * 2 * ZS * (YP - 1)], dt)
        m2yv = m2yc[:].rearrange("p (x h a b) -> p x h a b", x=2, h=2, a=ZS)
        nc.gpsimd.memset(m2yv[:, :, :, :, 0:2], SENT)
        nc.gpsimd.memset(m2yv[:, :, :, :, YP - 3:YP - 1], SENT)

        # 8-slot ring buffers for the streamed x-pass; slot j%8 holds plane j.
        # Rings make consecutive planes adjacent so x-ops batch pairwise.
        oyr = const.tile([128, 8 * VOX2], dt)
        m2xr = const.tile([128, 8 * VOX2], dt)
        m4xr = const.tile([128, 8 * VOX2], dt)

        def slot(t, j):
            return t[:, (j % 8) * VOX2:(j % 8 + 1) * VOX2]

        def pv(t, j):
            assert j % 8 <= 6
            return t[:, (j % 8) * VOX2:(j % 8 + 2) * VOX2]

        def oy_at(j):
            return neg[:] if (j < 0 or j >= NX) else slot(oyr, j)

        def m2x_at(j):
            if j < -1 or j >= NX:
                return neg[:]
            if j == -1:           # m2x[-1] = max(neg, oy[0]) = oy[0]
                return slot(oyr, 0)
            if j == NX - 1:       # m2x[63] = max(oy[63], neg) = oy[63]
                return slot(oyr, NX - 1)
            return slot(m2xr, j)

        def m4x_at(j):
            if j == -3:           # m4x[-3] = m2x[-1] = oy[0]
                return slot(oyr, 0)
            if j == -2:           # m4x[-2] = max(neg, m2x[0]) = m2x[0]
                return slot(m2xr, 0)
            if j == NX - 2:       # m4x[62] = max(m2x[62], neg) = m2x[62]
                return slot(m2xr, NX - 2)
            if j == NX - 1:       # m4x[63] = m2x[63] = oy[63]
                return slot(oyr, NX - 1)
            return slot(m4xr, j)

        w1v = w1sb[:].rearrange("p (h r) -> p h r", h=2)

        def v5(ap, h, a, b):
            return ap.rearrange("p (x h a b) -> p x h a b", x=2, h=h, a=a)

        p_ring = {}

        for g in range(NG + 2):
            if g < NG:
                # ---- single direct DMA: two dense transposed planes
                if g == 0:
                    P2 = P2_first
                else:
                    P2 = pp.tile([128, 2 * 2 * YZ], dt)
                    nc.sync.dma_start(
                        P2[:].rearrange("p (x h v) -> p x h v", x=2, h=2),
                        planes[g * 512:(g + 1) * 512, :].rearrange(
                            "(x h p) v -> p x h v", x=2, p=128),
                    )
                Pz = v5(P2[:], 2, ZH, D)   # [p, x, h, z, y] - y contiguous

                # ---- z-pass, 2 planes per op (window 7 over ZH -> ZS out);
                # group 0 runs per-plane so plane 0 starts during plane 1's DMA
                xsl = ((0, 1),) if g > 0 else ((0,), (1,))
                for xs_ in xsl:
                    xa, xb = xs_[0], xs_[-1] + 1
                    m2z = z2p.tile([128, (xb - xa) * 2 * (ZH - 1) * D], dt)
                    m2zv = m2z[:].rearrange("p (x h a b) -> p x h a b",
                                            x=xb - xa, h=2, a=ZH - 1)
                    nc.vector.tensor_max(m2zv, Pz[:, xa:xb, :, 0:ZH - 1, :],
                                         Pz[:, xa:xb, :, 1:ZH, :])
                    m4z = z4p.tile([128, (xb - xa) * 2 * (ZH - 3) * D], dt)
                    m4zv = m4z[:].rearrange("p (x h a b) -> p x h a b",
                                            x=xb - xa, h=2, a=ZH - 3)
                    nc.vector.tensor_max(m4zv, m2zv[:, :, :, 0:ZH - 3, :],
                                         m2zv[:, :, :, 2:ZH - 1, :])
                    nc.vector.tensor_max(
                        ypv[:, xa:xb, :, :, 3:3 + D], m4zv[:, :, :, 0:ZS, :],
                        m4zv[:, :, :, 3:3 + ZS, :]
                    )

                # ---- y-pass, 2 planes per op (y innermost, long runs);
                # only the varying cols [2:67) of m2y are computed
                nc.vector.tensor_max(m2yv[:, :, :, :, 2:YP - 3],
                                     ypv[:, :, :, :, 2:YP - 3],
                                     ypv[:, :, :, :, 3:YP - 2])
                m4y = y4p.tile([128, 2 * 2 * ZS * (YP - 3)], dt)
                m4yv = v5(m4y[:], 2, ZS, YP - 3)
                nc.vector.tensor_max(m4yv, m2yv[:, :, :, :, 0:YP - 3],
                                     m2yv[:, :, :, :, 2:YP - 1])
                oy2v = v5(pv(oyr, 2 * g), 2, ZS, D)
                nc.vector.tensor_max(oy2v, m4yv[:, :, :, :, 0:D],
                                     m4yv[:, :, :, :, 3:D + 3])
                p_ring[g] = P2

            # ---- x-pass: m2x pair {2g-1, 2g}, batched unless the ring
            # wraps (g%4==0) or we are at the stream edges
            if 1 <= g < NG and g % 4 != 0:
                nc.vector.tensor_max(pv(m2xr, 2 * g - 1), pv(oyr, 2 * g - 1),
                                     pv(oyr, 2 * g))
            else:
                for j in (2 * g - 1, 2 * g):
                    if 0 <= j < NX - 1:
                        nc.vector.tensor_max(slot(m2xr, j), oy_at(j),
                                             oy_at(j + 1))
            # m4x pair {2g-3, 2g-2}
            if 2 <= g <= 31 and g % 4 in (2, 3):
                nc.vector.tensor_max(pv(m4xr, 2 * g - 3), pv(m2xr, 2 * g - 3),
                                     pv(m2xr, 2 * g - 1))
            else:
                for j in (2 * g - 3, 2 * g - 2):
                    if -1 <= j < NX - 2:
                        nc.vector.tensor_max(slot(m4xr, j), m2x_at(j),
                                             m2x_at(j + 2))

            # ---- MLP + multiply for plane pair m (lag 2 groups)
            m = g - 2
            if 0 <= m < NG:
                px2 = pxp.tile([128, 2 * VOX2], dt)
                sg2 = sgp.tile([128, 2 * VOX2], dt)
                if m % 4 != 1 and 1 <= m <= 30:
                    nc.vector.tensor_max(px2[:], pv(m4xr, 2 * m - 3),
                                         pv(m4xr, 2 * m))
                else:
                    for xs in (0, 1):
                        k = 2 * m + xs
                        nc.vector.tensor_max(
                            px2[:, xs * VOX2:(xs + 1) * VOX2],
                            m4x_at(k - 3), m4x_at(k))
                for xs in (0, 1):
                    k = 2 * m + xs
                    px = px2[:, xs * VOX2:(xs + 1) * VOX2]
                    pxv = px.rearrange("p (h v) -> p h v", h=2)
                    hp = hpp.tile([128, VOXH], f32, space="PSUM")
                    for h in (0, 1):
                        nc.tensor.matmul(
                            hp[:], w1v[:, h, :], pxv[:, h, :],
                            start=(h == 0), stop=(h == 1)
                        )
                    hs = hsp.tile([128, VOXH], dt)
                    nc.scalar.activation(hs[:], hp[:], AF.Relu)
                    y2 = y2p.tile([128, VOX2], f32, space="PSUM")
                    y2v = y2[:].rearrange("p (h v) -> p h v", h=2)
                    for h in (0, 1):
                        nc.tensor.matmul(
                            y2v[:, h, :], w2sb[:, h * 128:(h + 1) * 128],
                            hs[:], start=True, stop=True,
                        )
                    nc.scalar.activation(sg2[:, xs * VOX2:(xs + 1) * VOX2],
                                         y2[:], AF.Sigmoid)

                # dense multiply by original feats planes (owned z only)
                P2m = p_ring.pop(m)
                Pko = v5(P2m[:], 2, ZH, D)[:, :, :, 3:3 + ZS, :]
                ot2 = otp.tile([128, 2 * VOX2], dt)
                nc.vector.tensor_mul(v5(ot2[:], 2, ZS, D),
                                     v5(sg2[:], 2, ZS, D), Pko)
                nc.scalar.dma_start(
                    outd[2 * m * 256:(2 * m + 2) * 256, :].rearrange(
                        "(x h p) v -> p x h v", x=2, p=128),
                    ot2[:].rearrange("p (x h v) -> p x h v", x=2, h=2),
                )

    nc.compile()
    return nc


def host_prep(cfg: Cfg, feats, coords, W1, W2):
    """Shard on host: build per-core dense transposed fp16 z-slab planes.
    Returns (in_maps, aux) where aux carries the coords for unshard."""
    D, ZS, ZH, NX = cfg.D, cfg.ZS, cfg.ZH, cfg.NX
    f16 = np.float16

    ix = coords[:, 0].astype(np.int64)
    iy = coords[:, 1].astype(np.int64)
    iz = coords[:, 2].astype(np.int64)

    feats16 = np.ascontiguousarray(feats.astype(f16))
    w1h = np.ascontiguousarray(W1.astype(f16))
    w2h = np.ascontiguousarray(W2.astype(f16))

    # global channel-major grid, z-major/y-minor, z padded by 3 each side
    Gp = np.full((D, 2, 128, D + 6, D), SENT, f16)
    Gp[ix, :, :, iz + 3, iy] = feats16.reshape(cfg.NPTS, 2, 128)

    in_maps = []
    for k in range(cfg.ncores):
        sl = np.ascontiguousarray(Gp[:, :, :, ZS * k:ZS * k + ZH, :])
        in_maps.append({
            "planes": sl.reshape(NX * 2 * 128, cfg.YZ),
            "w1": w1h,
            "w2": w2h,
        })
    return in_maps, (ix, iy, iz)


def unshard(cfg: Cfg, aux, results):
    """Gather occupied rows out of the dense per-core outputs (pure indexing)."""
    ix, iy, iz = aux
    out = np.empty((cfg.NPTS, C), np.float32)
    for k in range(cfg.ncores):
        o = np.asarray(results[k]["out"]).reshape(
            cfg.NX, 2, 128, cfg.ZS, cfg.D)
        m = (iz >= cfg.ZS * k) & (iz < cfg.ZS * (k + 1))
        out[m] = o[ix[m], :, :, iz[m] - cfg.ZS * k, iy[m]].reshape(
            -1, C).astype(np.float32)
    return out


_CACHE = {}


def _get_nc(cfg: Cfg):
    if cfg not in _CACHE:
        _CACHE[cfg] = build_nc(cfg)
    return _CACHE[cfg]


def kernel(feats, coords, W1, W2):
    from concourse.bass_utils import run_bass_kernel_spmd

    cfg = FULL
    nc = _get_nc(cfg)
    in_maps, aux = host_prep(
        cfg,
        np.asarray(feats, np.float32),
        np.asarray(coords),
        np.asarray(W1, np.float32),
        np.asarray(W2, np.float32),
    )
    res = run_bass_kernel_spmd(nc, in_maps, core_ids=list(range(cfg.ncores)))
    return unshard(cfg, aux, res.results)


# revision 40
# speedup vs baseline: 1.0003x; 1.0003x over previous
"""Trainium2 Bass kernel: sparse 7x7x7 stride-1 max-pool over a 64^3 voxel grid
(MinkowskiEngine semantics) + per-point MLP (1x1 conv -> ReLU -> 1x1 conv ->
sigmoid) * feats.

Strategy (8 NeuronCores, SPMD, no collectives):
  - Shard the dense grid along z: core k owns z in [8k, 8k+8), processes a
    14-plane z-slab (3-voxel halo each side, replicated -> no exchange).
  - HOST pre-builds the dense, channel-major (transposed) fp16 planes:
    planes[x] = [2, 128, 64*14] with empty voxels = SENT. The device does
    ZERO scatter / transpose / indirect-DMA work: x-planes arrive by direct
    DMA into SBUF already in [channel-part, (y,z)-free] layout.
  - Separable windowed max (7 = overlap of two 4-windows -> 3 tensor_max
    per axis), all on DVE in fp16 (2x mode; max is order-preserving).
    The z- and y-passes are batched 2 x-planes per instruction (5D APs) to
    amortize instruction overhead. GPSIMD is NOT used for elementwise work:
    its SBUF port is shared with DVE and halves DVE throughput.
  - Per-plane MLP without any transposes: hp = sum_h W1[h].T @ px[h] (R on
    partitions), then y2[h] = W2[:, h-block].T @ relu(hp) which lands
    directly back in [channel-part, vox] layout. Sigmoid on ACT.
  - out = P_orig * sig computed densely on DVE (fp16), written per plane
    pair; host gathers the occupied rows and casts to fp32 (pure unshard).

Measured on the 8-core axon TRN2 fleet: HW exec ~429 us (baseline 870-900 us),
rel err ~8.3e-4 vs the fp32 reference. DVE-bound: span = elems*0.52ns +
355 ops*146ns + ~24us framework warmup/drain, Vector engine at ~99% span
occupancy. Known remaining lever (~-7us, unimplemented): re-fetch the
owned-z slice from DRAM at multiply time instead of holding the P2 ring
(frees ~21KB SBUF), then batch the y-pass 4 planes per instruction.
"""

from contextlib import ExitStack
from dataclasses import dataclass

import numpy as np

C = 256
R = 128
SENT = -100.0  # < min(feats) ~ -5.6; keeps the fp16 MLP finite at empty voxels


@dataclass(frozen=True)
class Cfg:
    D: int = 64           # grid extent per axis
    ZS: int = 8           # owned z-planes per core
    NPTS: int = 100000    # total points
    ncores: int = 8

    @property
    def ZH(self):
        return self.ZS + 6

    @property
    def YZ(self):
        return self.D * self.ZH     # 896

    @property
    def NX(self):
        return self.D

    @property
    def VOXH(self):
        return self.D * self.ZS     # 512 owned voxels per x-plane

    @property
    def YP(self):
        return self.D + 6           # padded y extent


FULL = Cfg()


def build_nc(cfg: Cfg):
    """Build the (SPMD, per-core-identical) Bass program."""
    import concourse.bacc as bacc
    import concourse.tile as tile
    from concourse import mybir

    AF = mybir.ActivationFunctionType
    f32 = mybir.dt.float32
    dt = mybir.dt.float16

    D, ZS, ZH, YZ, NX = cfg.D, cfg.ZS, cfg.ZH, cfg.YZ, cfg.NX
    VOXH = cfg.VOXH
    VOX2 = 2 * VOXH           # 1024
    YP = cfg.YP
    NG = NX // 2              # 32 groups of 2 x-planes

    nc = bacc.Bacc("TRN2", target_bir_lowering=False, debug=False,
                   enable_asserts=False, num_devices=cfg.ncores)

    planes = nc.dram_tensor("planes", [NX * 2 * 128, YZ], dt,
                            kind="ExternalInput").ap()
    w1 = nc.dram_tensor("w1", [C, R], dt, kind="ExternalInput").ap()
    w2 = nc.dram_tensor("w2", [R, C], dt, kind="ExternalInput").ap()
    outd = nc.dram_tensor("out", [NX * 2 * 128, VOXH], dt,
                          kind="ExternalOutput").ap()

    with tile.TileContext(nc) as tc, ExitStack() as ctx:
        const = ctx.enter_context(tc.tile_pool(name="const", bufs=1))
        pp = ctx.enter_context(tc.tile_pool(name="pp", bufs=6))
        z2p = ctx.enter_context(tc.tile_pool(name="z2p", bufs=1))
        z4p = ctx.enter_context(tc.tile_pool(name="z4p", bufs=1))
        y4p = ctx.enter_context(tc.tile_pool(name="y4p", bufs=1))
        pxp = ctx.enter_context(tc.tile_pool(name="pxp", bufs=3))
        hpp = ctx.enter_context(tc.tile_pool(name="hpp", bufs=2, space="PSUM"))
        y2p = ctx.enter_context(tc.tile_pool(name="y2p", bufs=3, space="PSUM"))
        hsp = ctx.enter_context(tc.tile_pool(name="hsp", bufs=3))
        sgp = ctx.enter_context(tc.tile_pool(name="sgp", bufs=3))
        otp = ctx.enter_context(tc.tile_pool(name="otp", bufs=3))

        # ---- first plane pair prefetched (per-plane, so the z-pass can
        # start after half the transfer) ahead of everything else
        P2_first = pp.tile([128, 2 * 2 * YZ], dt)
        for x in (0, 1):
            nc.sync.dma_start(
                P2_first[:, x * 2 * YZ:(x + 1) * 2 * YZ].rearrange(
                    "p (h v) -> p h v", h=2),
                planes[x * 256:(x + 1) * 256, :].rearrange(
                    "(h p) v -> p h v", p=128),
            )

        # ---- constants
        w1sb = const.tile([128, 2 * R], dt)
        nc.sync.dma_start(
            w1sb[:].rearrange("p (h r) -> p h r", h=2),
            w1.rearrange("(h p) r -> p h r", p=128),
        )
        w2sb = const.tile([128, C], dt)
        nc.sync.dma_start(w2sb[:], w2)
        neg = const.tile([128, VOX2], dt)
        nc.gpsimd.memset(neg[:], SENT)
        # persistent y-padded buffer (2 planes wide, z-major); borders once
        ypad = const.tile([128, 2 * 2 * ZS * YP], dt)
        ypv = ypad[:].rearrange("p (x h a b) -> p x h a b", x=2, h=2, a=ZS)
        nc.gpsimd.memset(ypv[:, :, :, :, 0:3], SENT)
        nc.gpsimd.memset(ypv[:, :, :, :, D + 3:D + 6], SENT)
        # persistent m2y buffer: cols {0,1,67,68} are always SENT (both
        # window inputs are padding there) - memset once, never recompute
        m2yc = const.tile([128, 2 * 2 * ZS * (YP - 1)], dt)
        m2yv = m2yc[:].rearrange("p (x h a b) -> p x h a b", x=2, h=2, a=ZS)
        nc.gpsimd.memset(m2yv[:, :, :, :, 0:2], SENT)
        nc.gpsimd.memset(m2yv[:, :, :, :, YP - 3:YP - 1], SENT)

        # 8-slot ring buffers for the streamed x-pass; slot j%8 holds plane j.
        # Rings make consecutive planes adjacent so x-ops batch pairwise.
        oyr = const.tile([128, 8 * VOX2], dt)
        m2xr = const.tile([128, 8 * VOX2], dt)
        m4xr = const.tile([128, 8 * VOX2], dt)

        def slot(t, j):
            return t[:, (j % 8) * VOX2:(j % 8 + 1) * VOX2]

        def pv(t, j):
            assert j % 8 <= 6
            return t[:, (j % 8) * VOX2:(j % 8 + 2) * VOX2]

        def oy_at(j):
            return neg[:] if (j < 0 or j >= NX) else slot(oyr, j)

        def m2x_at(j):
            if j < -1 or j >= NX:
                return neg[:]
            if j == -1:           # m2x[-1] = max(neg, oy[0]) = oy[0]
                return slot(oyr, 0)
            if j == NX - 1:       # m2x[63] = max(oy[63], neg) = oy[63]
                return slot(oyr, NX - 1)
            return slot(m2xr, j)

        def m4x_at(j):
            if j == -3:           # m4x[-3] = m2x[-1] = oy[0]
                return slot(oyr, 0)
            if j == -2:           # m4x[-2] = max(neg, m2x[0]) = m2x[0]
                return slot(m2xr, 0)
            if j == NX - 2:       # m4x[62] = max(m2x[62], neg) = m2x[62]
                return slot(m2xr, NX - 2)
            if j == NX - 1:       # m4x[63] = m2x[63] = oy[63]
                return slot(oyr, NX - 1)
            return slot(m4xr, j)

        w1v = w1sb[:].rearrange("p (h r) -> p h r", h=2)

        def v5(ap, h, a, b):
            return ap.rearrange("p (x h a b) -> p x h a b", x=2, h=h, a=a)

        p_ring = {}

        for g in range(NG + 2):
            if g < NG:
                # ---- single direct DMA: two dense transposed planes
                if g == 0:
                    P2 = P2_first
                else:
                    P2 = pp.tile([128, 2 * 2 * YZ], dt)
                    nc.sync.dma_start(
                        P2[:].rearrange("p (x h v) -> p x h v", x=2, h=2),
                        planes[g * 512:(g + 1) * 512, :].rearrange(
                            "(x h p) v -> p x h v", x=2, p=128),
                    )
                Pz = v5(P2[:], 2, ZH, D)   # [p, x, h, z, y] - y contiguous

                # ---- z-pass, 2 planes per op (window 7 over ZH -> ZS out);
                # group 0 runs per-plane so plane 0 starts during plane 1's DMA
                xsl = ((0, 1),) if g > 0 else ((0,), (1,))
                for xs_ in xsl:
                    xa, xb = xs_[0], xs_[-1] + 1
                    m2z = z2p.tile([128, (xb - xa) * 2 * (ZH - 1) * D], dt)
                    m2zv = m2z[:].rearrange("p (x h a b) -> p x h a b",
                                            x=xb - xa, h=2, a=ZH - 1)
                    nc.vector.tensor_max(m2zv, Pz[:, xa:xb, :, 0:ZH - 1, :],
                                         Pz[:, xa:xb, :, 1:ZH, :])
                    m4z = z4p.tile([128, (xb - xa) * 2 * (ZH - 3) * D], dt)
                    m4zv = m4z[:].rearrange("p (x h a b) -> p x h a b",
                                            x=xb - xa, h=2, a=ZH - 3)
                    nc.vector.tensor_max(m4zv, m2zv[:, :, :, 0:ZH - 3, :],
                                         m2zv[:, :, :, 2:ZH - 1, :])
                    nc.vector.tensor_max(
                        ypv[:, xa:xb, :, :, 3:3 + D], m4zv[:, :, :, 0:ZS, :],
                        m4zv[:, :, :, 3:3 + ZS, :]
                    )

                # ---- y-pass, 2 planes per op (y innermost, long runs);
                # only the varying cols [2:67) of m2y are computed
                nc.vector.tensor_max(m2yv[:, :, :, :, 2:YP - 3],
                                     ypv[:, :, :, :, 2:YP - 3],
                                     ypv[:, :, :, :, 3:YP - 2])
                m4y = y4p.tile([128, 2 * 2 * ZS * (YP - 3)], dt)
                m4yv = v5(m4y[:], 2, ZS, YP - 3)
                nc.vector.tensor_max(m4yv, m2yv[:, :, :, :, 0:YP - 3],
                                     m2yv[:, :, :, :, 2:YP - 1])
                oy2v = v5(pv(oyr, 2 * g), 2, ZS, D)
                nc.vector.tensor_max(oy2v, m4yv[:, :, :, :, 0:D],
                                     m4yv[:, :, :, :, 3:D + 3])
                p_ring[g] = P2

            # ---- x-pass: m2x pair {2g-1, 2g}, batched unless the ring
            # wraps (g%4==0) or we are at the stream edges
            if 1 <= g < NG and g % 4 != 0:
                nc.vector.tensor_max(pv(m2xr, 2 * g - 1), pv(oyr, 2 * g - 1),
                                     pv(oyr, 2 * g))
            else:
                for j in (2 * g - 1, 2 * g):
                    if 0 <= j < NX - 1:
                        nc.vector.tensor_max(slot(m2xr, j), oy_at(j),
                                             oy_at(j + 1))
            # m4x pair {2g-3, 2g-2}
            if 2 <= g <= 31 and g % 4 in (2, 3):
                nc.vector.tensor_max(pv(m4xr, 2 * g - 3), pv(m2xr, 2 * g - 3),
                                     pv(m2xr, 2 * g - 1))
            else:
                for j in (2 * g - 3, 2 * g - 2):
                    if -1 <= j < NX - 2:
                        nc.vector.tensor_max(slot(m4xr, j), m2x_at(j),
                                             m2x_at(j + 2))

            # ---- MLP + multiply for plane pair m (lag 2 groups)
            m = g - 2
            if 0 <= m < NG:
                px2 = pxp.tile([128, 2 * VOX2], dt)
                sg2 = sgp.tile([128, 2 * VOX2], dt)
                if m % 4 != 1 and 1 <= m <= 30:
                    nc.vector.tensor_max(px2[:], pv(m4xr, 2 * m - 3),
                                         pv(m4xr, 2 * m))
                else:
                    for xs in (0, 1):
                        k = 2 * m + xs
                        nc.vector.tensor_max(
                            px2[:, xs * VOX2:(xs + 1) * VOX2],
                            m4x_at(k - 3), m4x_at(k))
                for xs in (0, 1):
                    k = 2 * m + xs
                    px = px2[:, xs * VOX2:(xs + 1) * VOX2]
                    pxv = px.rearrange("p (h v) -> p h v", h=2)
                    hp = hpp.tile([128, VOXH], f32, space="PSUM")
                    for h in (0, 1):
                        nc.tensor.matmul(
                            hp[:], w1v[:, h, :], pxv[:, h, :],
                            start=(h == 0), stop=(h == 1)
                        )
                    hs = hsp.tile([128, VOXH], dt)
                    nc.scalar.activation(hs[:], hp[:], AF.Relu)
                    y2 = y2p.tile([128, VOX2], f32, space="PSUM")
                    y2v = y2[:].rearrange("p (h v) -> p h v", h=2)
                    for h in (0, 1):
                        nc.tensor.matmul(
                            y2v[:, h, :], w2sb[:, h * 128:(h + 1) * 128],
                            hs[:], start=True, stop=True,
                        )
                    nc.scalar.activation(sg2[:, xs * VOX2:(xs + 1) * VOX2],
                                         y2[:], AF.Sigmoid)

                # dense multiply by original feats planes (owned z only)
                P2m = p_ring.pop(m)
                Pko = v5(P2m[:], 2, ZH, D)[:, :, :, 3:3 + ZS, :]
                ot2 = otp.tile([128, 2 * VOX2], dt)
                nc.vector.tensor_mul(v5(ot2[:], 2, ZS, D),
                                     v5(sg2[:], 2, ZS, D), Pko)
                nc.scalar.dma_start(
                    outd[2 * m * 256:(2 * m + 2) * 256, :].rearrange(
                        "(x h p) v -> p x h v", x=2, p=128),
                    ot2[:].rearrange("p (x h v) -> p x h v", x=2, h=2),
                )

    nc.compile()
    return nc


def host_prep(cfg: Cfg, feats, coords, W1, W2):
    """Shard on host: build per-core dense transposed fp16 z-slab planes.
    Returns (in_maps, aux) where aux carries the coords for unshard."""
    D, ZS, ZH, NX = cfg.D, cfg.ZS, cfg.ZH, cfg.NX
    f16 = np.float16

    ix = coords[:, 0].astype(np.int64)
    iy = coords[:, 1].astype(np.int64)
    iz = coords[:, 2].astype(np.int64)

    feats16 = np.ascontiguousarray(feats.astype(f16))
    w1h = np.ascontiguousarray(W1.astype(f16))
    w2h = np.ascontiguousarray(W2.astype(f16))

    # global channel-major grid, z-major/y-minor, z padded by 3 each side
    Gp = np.full((D, 2, 128, D + 6, D), SENT, f16)
    Gp[ix, :, :, iz + 3, iy] = feats16.reshape(cfg.NPTS, 2, 128)

    in_maps = []
    for k in range(cfg.ncores):
        sl = np.ascontiguousarray(Gp[:, :, :, ZS * k:ZS * k + ZH, :])
        in_maps.append({
            "planes": sl.reshape(NX * 2 * 128, cfg.YZ),
            "w1": w1h,
            "w2": w2h,
        })
    return in_maps, (ix, iy, iz)


def unshard(cfg: Cfg, aux, results):
    """Gather occupied rows out of the dense per-core outputs (pure indexing)."""
    ix, iy, iz = aux
    out = np.empty((cfg.NPTS, C), np.float32)
    for k in range(cfg.ncores):
        o = np.asarray(results[k]["out"]).reshape(
            cfg.NX, 2, 128, cfg.ZS, cfg.D)
        m = (iz >= cfg.ZS * k) & (iz < cfg.ZS * (k + 1))
        out[m] = o[ix[m], :, :, iz[m] - cfg.ZS * k, iy[m]].reshape(
            -1, C).astype(np.float32)
    return out


_CACHE = {}


def _get_nc(cfg: Cfg):
    if cfg not in _CACHE:
        _CACHE[cfg] = build_nc(cfg)
    return _CACHE[cfg]


def kernel(feats, coords, W1, W2):
    from concourse.bass_utils import run_bass_kernel_spmd

    cfg = FULL
    nc = _get_nc(cfg)
    in_maps, aux = host_prep(
        cfg,
        np.asarray(feats, np.float32),
        np.asarray(coords),
        np.asarray(W1, np.float32),
        np.asarray(W2, np.float32),
    )
    res = run_bass_kernel_spmd(nc, in_maps, core_ids=list(range(cfg.ncores)))
    return unshard(cfg, aux, res.results)
